# revision 15
# baseline (speedup 1.0000x reference)
"""Trainium2 Bass kernel for an attention layer whose math collapses.

The module computes softmax over a size-1 axis, so the attention
weights are exactly 1.0 and the output is context[b,0,d] = sum_t
a[b,t,d]. The MLP branch (W1, b1, W2, b2) and s_prev never affect the
output. Pure data parallel: each of 8 cores reduces its [16, 512, 512]
shard over time. Memory-bound: ~16 MiB HBM read per core; the read
stream runs at the HBM/fabric cap (trace-measured ~420 GB/s, dipping
to the ~358 GB/s HBM fair-share when the paired core's phase collides)
-> ~40-48 us byte stream + fixed framework pre/postamble + short tail.

Design (chosen against perfetto traces of 6 prior variants):
  - SWDGE cast-DMAs: fp32 -> bf16 conversion happens inside the DMA
    datapath (GPSIMD-issued software DGE), so SBUF receives half the
    bytes and no compute engine touches the data before the PE. This
    beat (a) DVE fold-adds (GpSimd/DVE tensor ops cross-stall 2-4x on
    the shared SBUF port), (b) DVE tensor_copy casts (an extra pipeline
    stage whose scratch-slot recycling couples DVE to PE progress), and
    (c) an in-place bitcast cast (data race, wrong results).
  - Hybrid head-start: slabs 0 and 1 load as plain fp32 over the two
    otherwise-idle HWDGE rings (first byte ~0.6 us vs SWDGE's ~1 us,
    and three DGE paths fill the 16 SDMA engines during the ramp);
    DVE casts just those two slabs to bf16.
  - No wait on store completion: the last 2 KiB store's HBM-write
    receipt (~1.2-1.8 us measured) lands well inside the ~7 us walrus
    postamble that runs before NEFF completion; waiting for it held
    the block-exit barrier on Sync for the whole receipt round trip.
  - 14x 1 MiB cast-DMAs (one batch each; every transfer spans all 128
    partitions - 32-partition transfers only reach 8 of 16 SBUF AXI
    ports and halve the stream rate) + a half/quarter/quarter endgame
    per last batch, so the post-last-byte chain is one N=512 matmul +
    psum bounce + 2 KiB store. Items are consumed in single-queue FIFO
    arrival order (batch 14's pieces fully precede batch 15's).
  - PE reduces each batch with 4 accumulating bf16 matmuls against the
    preamble's constant bf16 ones [128, 1] (fp32 PSUM accumulation).
    bf16 keeps the PE at ~0.4-0.6 us per matmul (fp32 would LOW/HIGH
    split to ~2 us) and costs ~5e-4 rel err vs the 2e-2 gate. Eight
    psum banks hold two batch rows each at partition offsets {0, 32}.
  - ACT bounces each finished psum row to SBUF (DMA cannot read PSUM);
    per-batch 2 KiB stores on the SP HWDGE ring overlap all but the
    last store's receipt. All stores count one shared semaphore; the
    final wait needs the exact total 16*16 (order-independent).
  - Per-load-DMA completion semaphores are required: concurrent DMA
    completions are unordered across the 16 SDMA engines.
  - The ~7 us end-of-kernel postamble (walrus resets all 256 HW
    semaphores, serialized per engine, inside the measured window) and
    the ~1 us boot-to-first-issue preamble are framework-fixed.
"""

from contextlib import ExitStack

import numpy as np

B, TX, D = 128, 512, 512
N_CORES = 8
NB = B // N_CORES   # 16 batches per core
P = 128             # SBUF partitions
NSLAB = 16
FPP = NB * TX * D // (NSLAB * P)  # f32 per partition per slab = 2048

_CACHE: dict = {}


def _build_bass():
    import concourse.bass as bass
    import concourse.mybir as mybir

    f32 = mybir.dt.float32
    bf16 = mybir.dt.bfloat16
    nc = bass.Bass("TRN2")
    a = nc.dram_tensor("a", [NB, TX, D], f32, kind="ExternalInput")
    out = nc.dram_tensor("out", [NB, D], f32, kind="ExternalOutput")

    ones = nc.const_aps.aps[(bf16, 1.0)]
    a_sl = a.rearrange("b t d -> (b t d)").rearrange(
        "(g p f) -> g p f", g=NSLAB, p=P
    )

    with ExitStack() as ctx:
        bbuf = ctx.enter_context(nc.sbuf_tensor([P, NSLAB * FPP], bf16))
        # fp32 staging for the two HWDGE head-start slabs (0 and 1).
        abuf = ctx.enter_context(nc.sbuf_tensor([P, 2 * FPP], f32))
        ost = ctx.enter_context(nc.sbuf_tensor([1, NB * D], f32))
        psb = [
            ctx.enter_context(nc.psum_tensor(f"ps{i}", [64, D], f32))
            for i in range(8)
        ]
        ld_sems = [
            ctx.enter_context(nc.semaphore(f"ld_sem{g}"))
            for g in range(NSLAB - 2)
        ]
        vq = ctx.enter_context(nc.semaphore("vq"))
        eg_sems = {
            (g, q): ctx.enter_context(nc.semaphore(f"eg{g}_{q}"))
            for g in (14, 15)
            for q in (0, 1, 2)
        }
        pe_sem = ctx.enter_context(nc.semaphore("pe_sem"))
        cp_sem = ctx.enter_context(nc.semaphore("cp_sem"))
        st_sem = ctx.enter_context(nc.semaphore("st_sem"))
        block = ctx.enter_context(nc.Block(no_gpsimd_drain=True))

        bbuf_t = bbuf[:].rearrange("p (g f) -> p g f", g=NSLAB)
        abuf_t = abuf[:].rearrange("p (g f) -> p g f", g=2)
        HF = FPP // 2   # 1024
        QF = FPP // 4   # 512
        EG_OFF = {0: (0, HF), 1: (HF, QF), 2: (HF + QF, QF)}
        a_q = a.rearrange("b t d -> (b t d)").rearrange(
            "(x p f) -> x p f", x=4 * NSLAB, p=P
        )
        a_hl = a.rearrange("b t d -> (b t d)").rearrange(
            "(x p f) -> x p f", x=2 * NSLAB, p=P
        )
        # Endgame items in single-queue FIFO arrival order: all of batch
        # 14's pieces land before batch 15's, so batch 14's bounce and
        # store fully overlap batch 15's matmuls.
        ITEM_ORDER = list(range(14)) + [
            (14, 0), (14, 1), (14, 2), (15, 0), (15, 1), (15, 2)
        ]

        @block.gpsimd
        def _(gpsimd):
            # Slabs 0 and 1 go over the HWDGE rings (faster first byte;
            # both DGE paths fill the 16 SDMA engines during the ramp).
            for g in range(2, NSLAB - 2):
                gpsimd.dma_start(out=bbuf_t[:, g], in_=a_sl[g]).then_inc(
                    ld_sems[g], 16
                )
            for g in (14, 15):
                gpsimd.dma_start(
                    out=bbuf_t[:, g, 0:HF], in_=a_hl[2 * g]
                ).then_inc(eg_sems[(g, 0)], 16)
                for q in (1, 2):
                    o, n = EG_OFF[q]
                    gpsimd.dma_start(
                        out=bbuf_t[:, g, o : o + n], in_=a_q[4 * g + 1 + q]
                    ).then_inc(eg_sems[(g, q)], 16)

        @block.sync
        def _(sync):
            # HWDGE head-start: slab 0 (fp32) while SWDGE spins up.
            sync.dma_start(out=abuf_t[:, 0], in_=a_sl[0]).then_inc(
                ld_sems[0], 16
            )
            for g in range(NSLAB):
                sync.wait_ge(cp_sem, g + 1)
                sync.dma_start(
                    out=out[g : g + 1, :], in_=ost[0:1, g * D : (g + 1) * D]
                ).then_inc(st_sem, 16)
            # No wait on st_sem: the last 2 KiB store's HBM-write receipt
            # (~1.2-1.8 us measured) lands well inside the ~7 us walrus
            # postamble that runs before NEFF completion, and nothing in
            # this or a subsequent execution reads the output region
            # before then. Waiting here would hold the block-exit
            # barrier (Sync is the straggler) for the receipt round trip.

        @block.scalar
        def _(scalar):
            # HWDGE head-start: slab 1 on the second ring.
            scalar.dma_start(out=abuf_t[:, 1], in_=a_sl[1]).then_inc(
                ld_sems[1], 16
            )
            for g in range(NSLAB):
                off = 32 * (g % 2)
                n_items = (g + 1) if g < 14 else (17 if g == 14 else 20)
                scalar.wait_ge(pe_sem, n_items)
                scalar.copy(
                    ost[:, g * D : (g + 1) * D], psb[g // 2][off : off + 1, :]
                ).then_inc(cp_sem, 1)

        @block.vector
        def _(vector):
            # Cast the two HWDGE head-start slabs. The 2-port DVE copy
            # briefly locks GPSIMD out of the shared SBUF port (SWDGE
            # descriptor emission pauses ~1.2 us per cast) but emission
            # finishes ~25 us before the stream needs it.
            for g in (0, 1):
                vector.wait_ge(ld_sems[g], 16)
                vector.tensor_copy(bbuf_t[:, g], abuf_t[:, g]).then_inc(vq, 1)

        @block.tensor
        def _(tensor):
            for item in ITEM_ORDER:
                if isinstance(item, int):
                    g = item
                    if g < 2:
                        tensor.wait_ge(vq, g + 1)
                    else:
                        tensor.wait_ge(ld_sems[g], 16)
                    pieces = [(j * D, j == 0, j == 3) for j in range(4)]
                else:
                    g, q = item
                    tensor.wait_ge(eg_sems[(g, q)], 16)
                    o, _n = EG_OFF[q]
                    if q == 0:
                        pieces = [(0, True, False), (D, False, False)]
                    else:
                        pieces = [(o, False, q == 2)]
                off = 32 * (g % 2)
                for o, first, last in pieces:
                    mm = tensor.matmul(
                        psb[g // 2][off : off + 1, :],
                        lhsT=ones[:, 0:1],
                        rhs=bbuf_t[:, g, o : o + D],
                        start=first,
                        stop=last,
                    )
                mm.then_inc(pe_sem, 1)

    return nc


def _get_bass():
    if "nc" not in _CACHE:
        _CACHE["nc"] = _build_bass()
    return _CACHE["nc"]


def run_spmd(a, **spmd_kwargs):
    from concourse.bass_utils import run_bass_kernel_spmd

    nc = _get_bass()
    a = np.ascontiguousarray(np.asarray(a), dtype=np.float32)
    assert a.shape == (B, TX, D), a.shape
    in_maps = [{"a": a[k * NB : (k + 1) * NB]} for k in range(N_CORES)]
    res = run_bass_kernel_spmd(nc, in_maps, list(range(N_CORES)), **spmd_kwargs)
    out = np.concatenate([res.results[k]["out"] for k in range(N_CORES)], axis=0)
    return out.reshape(B, 1, D).astype(np.float32), res


def kernel(a, s_prev=None, W1=None, b1=None, W2=None, b2=None, **_unused):
    out, _ = run_spmd(a)
    return out


# revision 20
# speedup vs baseline: 1.0632x; 1.0632x over previous
"""Trainium2 Bass kernel for an attention layer whose math collapses.

The module computes softmax over a size-1 axis, so the attention
weights are exactly 1.0 and the output is context[b,0,d] = sum_t
a[b,t,d]. The MLP branch (W1, b1, W2, b2) and s_prev never affect the
output. Pure data parallel: each of 8 cores reduces its [16, 512, 512]
shard over time. Memory-bound: ~16 MiB HBM read per core; the read
stream runs at the HBM/fabric cap (trace-measured ~420 GB/s, dipping
to the ~358 GB/s HBM fair-share when the paired core's phase collides)
-> ~40-48 us byte stream + fixed framework pre/postamble + short tail.

Design (chosen against perfetto traces of 6 prior variants):
  - SWDGE cast-DMAs: fp32 -> bf16 conversion happens inside the DMA
    datapath (GPSIMD-issued software DGE), so SBUF receives half the
    bytes and no compute engine touches the data before the PE. This
    beat (a) DVE fold-adds (GpSimd/DVE tensor ops cross-stall 2-4x on
    the shared SBUF port), (b) DVE tensor_copy casts (an extra pipeline
    stage whose scratch-slot recycling couples DVE to PE progress), and
    (c) an in-place bitcast cast (data race, wrong results).
  - Hybrid head-start: slabs 0 and 1 load as plain fp32 over the two
    otherwise-idle HWDGE rings (first byte ~0.6 us vs SWDGE's ~1 us,
    and three DGE paths fill the 16 SDMA engines during the ramp);
    DVE casts just those two slabs to bf16.
  - No wait on store completion: the last 2 KiB store's HBM-write
    receipt (~1.2-1.8 us measured) lands well inside the ~7 us walrus
    postamble that runs before NEFF completion; waiting for it held
    the block-exit barrier on Sync for the whole receipt round trip.
  - 14x 1 MiB cast-DMAs (one batch each; every transfer spans all 128
    partitions - 32-partition transfers only reach 8 of 16 SBUF AXI
    ports and halve the stream rate) + a half/quarter/quarter endgame
    per last batch, so the post-last-byte chain is one N=512 matmul +
    psum bounce + 2 KiB store. Items are consumed in single-queue FIFO
    arrival order (batch 14's pieces fully precede batch 15's).
  - PE reduces each batch with 4 accumulating bf16 matmuls against the
    preamble's constant bf16 ones [128, 1] (fp32 PSUM accumulation).
    bf16 keeps the PE at ~0.4-0.6 us per matmul (fp32 would LOW/HIGH
    split to ~2 us) and costs ~5e-4 rel err vs the 2e-2 gate. Eight
    psum banks hold two batch rows each at partition offsets {0, 32}.
  - ACT bounces each finished psum row to SBUF (DMA cannot read PSUM)
    and issues the batch's 2 KiB store itself right after (program
    order on one sequencer replaces a cross-engine sem hop; a sem
    handshake still guards the same-engine bounce-write -> store-read
    RAW, which has no hardware interlock).
  - Per-load-DMA completion semaphores are required: concurrent DMA
    completions are unordered across the 16 SDMA engines.
  - The ~7 us end-of-kernel postamble (walrus resets all 256 HW
    semaphores, serialized per engine, inside the measured window) and
    the ~1 us boot-to-first-issue preamble are framework-fixed.
"""

from contextlib import ExitStack

import numpy as np

B, TX, D = 128, 512, 512
N_CORES = 8
NB = B // N_CORES   # 16 batches per core
P = 128             # SBUF partitions
NSLAB = 16
FPP = NB * TX * D // (NSLAB * P)  # f32 per partition per slab = 2048

_CACHE: dict = {}


def _build_bass():
    import concourse.bass as bass
    import concourse.mybir as mybir

    f32 = mybir.dt.float32
    bf16 = mybir.dt.bfloat16
    nc = bass.Bass("TRN2")
    a = nc.dram_tensor("a", [NB, TX, D], f32, kind="ExternalInput")
    out = nc.dram_tensor("out", [NB, D], f32, kind="ExternalOutput")

    ones = nc.const_aps.aps[(bf16, 1.0)]
    a_sl = a.rearrange("b t d -> (b t d)").rearrange(
        "(g p f) -> g p f", g=NSLAB, p=P
    )

    with ExitStack() as ctx:
        bbuf = ctx.enter_context(nc.sbuf_tensor([P, NSLAB * FPP], bf16))
        # fp32 staging for the two HWDGE head-start slabs (0 and 1).
        abuf = ctx.enter_context(nc.sbuf_tensor([P, 2 * FPP], f32))
        ost = ctx.enter_context(nc.sbuf_tensor([1, NB * D], f32))
        psb = [
            ctx.enter_context(nc.psum_tensor(f"ps{i}", [64, D], f32))
            for i in range(8)
        ]
        ld_sems = [
            ctx.enter_context(nc.semaphore(f"ld_sem{g}"))
            for g in range(NSLAB - 2)
        ]
        vq = ctx.enter_context(nc.semaphore("vq"))
        eg_sems = {
            (g, q): ctx.enter_context(nc.semaphore(f"eg{g}_{q}"))
            for g in (14, 15)
            for q in (0, 1, 2)
        }
        pe_sem = ctx.enter_context(nc.semaphore("pe_sem"))
        cp_sem = ctx.enter_context(nc.semaphore("cp_sem"))
        # Stores inc this but nothing waits on it: walrus requires HWDGE
        # dynamic DMAs to carry sync info (a completion semaphore).
        st_sem = ctx.enter_context(nc.semaphore("st_sem"))
        block = ctx.enter_context(nc.Block(no_gpsimd_drain=True))

        bbuf_t = bbuf[:].rearrange("p (g f) -> p g f", g=NSLAB)
        abuf_t = abuf[:].rearrange("p (g f) -> p g f", g=2)
        HF = FPP // 2   # 1024
        QF = FPP // 4   # 512
        EG_OFF = {0: (0, HF), 1: (HF, QF), 2: (HF + QF, QF)}
        a_q = a.rearrange("b t d -> (b t d)").rearrange(
            "(x p f) -> x p f", x=4 * NSLAB, p=P
        )
        a_hl = a.rearrange("b t d -> (b t d)").rearrange(
            "(x p f) -> x p f", x=2 * NSLAB, p=P
        )
        # Endgame items in single-queue FIFO arrival order: all of batch
        # 14's pieces land before batch 15's, so batch 14's bounce and
        # store fully overlap batch 15's matmuls.
        ITEM_ORDER = list(range(14)) + [
            (14, 0), (14, 1), (14, 2), (15, 0), (15, 1), (15, 2)
        ]

        @block.gpsimd
        def _(gpsimd):
            # Slabs 0 and 1 go over the HWDGE rings (faster first byte;
            # both DGE paths fill the 16 SDMA engines during the ramp).
            for g in range(2, NSLAB - 2):
                gpsimd.dma_start(out=bbuf_t[:, g], in_=a_sl[g]).then_inc(
                    ld_sems[g], 16
                )
            for g in (14, 15):
                gpsimd.dma_start(
                    out=bbuf_t[:, g, 0:HF], in_=a_hl[2 * g]
                ).then_inc(eg_sems[(g, 0)], 16)
                for q in (1, 2):
                    o, n = EG_OFF[q]
                    gpsimd.dma_start(
                        out=bbuf_t[:, g, o : o + n], in_=a_q[4 * g + 1 + q]
                    ).then_inc(eg_sems[(g, q)], 16)

        @block.sync
        def _(sync):
            # HWDGE head-start: slab 0 (fp32) while SWDGE spins up.
            sync.dma_start(out=abuf_t[:, 0], in_=a_sl[0]).then_inc(
                ld_sems[0], 16
            )

        @block.scalar
        def _(scalar):
            # HWDGE head-start: slab 1 on the second ring.
            scalar.dma_start(out=abuf_t[:, 1], in_=a_sl[1]).then_inc(
                ld_sems[1], 16
            )
            # Bounce each finished psum row to SBUF and immediately issue
            # its 2 KiB store on the same (ACT) sequencer: program order
            # replaces the old cp_sem cross-engine hop to Sync. No wait
            # on store completion: the last store's HBM-write receipt
            # (~1.2-1.8 us measured) lands well inside the ~7 us walrus
            # postamble that runs before NEFF completion, and nothing in
            # this or a subsequent execution reads the output region
            # before then.
            for g in range(NSLAB):
                off = 32 * (g % 2)
                n_items = (g + 1) if g < 14 else (17 if g == 14 else 20)
                scalar.wait_ge(pe_sem, n_items)
                # Same-engine RAW (bounce write -> store read) has no HW
                # interlock; the sem fires at ACTIVATE completion.
                scalar.copy(
                    ost[:, g * D : (g + 1) * D], psb[g // 2][off : off + 1, :]
                ).then_inc(cp_sem, 1)
                scalar.wait_ge(cp_sem, g + 1)
                scalar.dma_start(
                    out=out[g : g + 1, :], in_=ost[0:1, g * D : (g + 1) * D]
                ).then_inc(st_sem, 16)

        @block.vector
        def _(vector):
            # Cast the two HWDGE head-start slabs. The 2-port DVE copy
            # briefly locks GPSIMD out of the shared SBUF port (SWDGE
            # descriptor emission pauses ~1.2 us per cast) but emission
            # finishes ~25 us before the stream needs it.
            for g in (0, 1):
                vector.wait_ge(ld_sems[g], 16)
                vector.tensor_copy(bbuf_t[:, g], abuf_t[:, g]).then_inc(vq, 1)

        @block.tensor
        def _(tensor):
            for item in ITEM_ORDER:
                if isinstance(item, int):
                    g = item
                    if g < 2:
                        tensor.wait_ge(vq, g + 1)
                    else:
                        tensor.wait_ge(ld_sems[g], 16)
                    pieces = [(j * D, j == 0, j == 3) for j in range(4)]
                else:
                    g, q = item
                    tensor.wait_ge(eg_sems[(g, q)], 16)
                    o, _n = EG_OFF[q]
                    if q == 0:
                        pieces = [(0, True, False), (D, False, False)]
                    else:
                        pieces = [(o, False, q == 2)]
                off = 32 * (g % 2)
                for o, first, last in pieces:
                    mm = tensor.matmul(
                        psb[g // 2][off : off + 1, :],
                        lhsT=ones[:, 0:1],
                        rhs=bbuf_t[:, g, o : o + D],
                        start=first,
                        stop=last,
                    )
                mm.then_inc(pe_sem, 1)

    return nc


def _get_bass():
    if "nc" not in _CACHE:
        _CACHE["nc"] = _build_bass()
    return _CACHE["nc"]


def run_spmd(a, **spmd_kwargs):
    from concourse.bass_utils import run_bass_kernel_spmd

    nc = _get_bass()
    a = np.ascontiguousarray(np.asarray(a), dtype=np.float32)
    assert a.shape == (B, TX, D), a.shape
    in_maps = [{"a": a[k * NB : (k + 1) * NB]} for k in range(N_CORES)]
    res = run_bass_kernel_spmd(nc, in_maps, list(range(N_CORES)), **spmd_kwargs)
    out = np.concatenate([res.results[k]["out"] for k in range(N_CORES)], axis=0)
    return out.reshape(B, 1, D).astype(np.float32), res


def kernel(a, s_prev=None, W1=None, b1=None, W2=None, b2=None, **_unused):
    out, _ = run_spmd(a)
    return out
